# revision 1
# baseline (speedup 1.0000x reference)
"""Trainium2 Bass kernel for nn_ContextEmbedding (cross-attention context embedding).

Reference math (per batch b):
    Q = E @ q_w.T        [256, 1024]
    K = I @ k_w.T        [4096, 1024]
    V = I @ v_w.T        [4096, 1024]
    S_h = Q_h @ K_h.T    per head (16 heads, head_dim 64)
    P = softmax(S, -1)
    U_h = P_h @ V_h
    O = (U @ p_w.T);  O /= ||O||_2(row)
    out = concat([E, O], -1)   [256, 2048]

Sharding: pure data-parallel over batch B=8 across the 8 NeuronCores (one
batch per core, no collectives). Host pre-transposes/casts activations and
weights to bf16 so every matmul has its contraction dim on SBUF partitions,
and re-assembles the output (E-passthrough concat happens on host).

Per-core dataflow (all matmuls bf16 with f32 PSUM accumulation):
  Q^T [1024,256] and K^T [1024,4096] in o-on-partitions layout, so partition
  tile j holds head pair (2j, 2j+1) in rows 0:64 / 64:128 -> QK^T runs as
  concurrent row-group-tiled matmuls producing S^T [m2, n1]. exp() on ScalarE
  (PSUM->SBUF, 1024-wide ops). V in natural [m2, o] layout with a ones column
  appended per head (65-wide stationary) so AV yields U^T rows 0:64 plus the
  softmax row-sum in row 64. Division by the row-sum uses a ones-matmul
  partition broadcast. P-projection consumes U^T directly; the final L2 norm
  reduces over partitions with a ones-matmul and applies 1/sqrt via
  exp(-0.5*ln(x)) on ScalarE.
"""

import os

import numpy as np
import ml_dtypes

B, N1, N2, D = 8, 256, 4096, 1024
H, HD = 16, 64
PAIRS = H // 2  # 8 partition-tiles of head pairs
N_CORES = 8

BF16 = ml_dtypes.bfloat16

_COMPILED = None  # (nc,) cache so repeated kernel() calls skip the rebuild
LAST_RESULT = None  # BassKernelResults of the most recent run (for harnesses)


def _build():
    import concourse.bacc as bacc
    import concourse.mybir as mybir
    from concourse import tile
    from contextlib import ExitStack

    _ph = os.environ.get("KERNEL_PHASE", "5")
    phase = int(_ph[0])
    sub = _ph[1:]

    dt = mybir.dt
    Exp = mybir.ActivationFunctionType.Exp
    Ln = mybir.ActivationFunctionType.Ln

    nc = bacc.Bacc("TRN2", target_bir_lowering=False, debug=False,
                   num_devices=N_CORES)

    # ---- per-core DRAM tensors (host pre-transposed / pre-cast) ----
    it_d = nc.dram_tensor("it", [D, N2], dt.bfloat16, kind="ExternalInput")
    et_d = nc.dram_tensor("et", [D, N1], dt.bfloat16, kind="ExternalInput")
    qw_d = nc.dram_tensor("qw", [D, D], dt.bfloat16, kind="ExternalInput")
    kw_d = nc.dram_tensor("kw", [D, D], dt.bfloat16, kind="ExternalInput")
    vw_d = nc.dram_tensor("vw", [D, D], dt.bfloat16, kind="ExternalInput")
    pw_d = nc.dram_tensor("pw", [D, D], dt.bfloat16, kind="ExternalInput")
    qb_d = nc.dram_tensor("qb", [128, 8], dt.float32, kind="ExternalInput")
    kb_d = nc.dram_tensor("kb", [128, 8], dt.float32, kind="ExternalInput")
    pb_d = nc.dram_tensor("pb", [128, 8], dt.float32, kind="ExternalInput")
    vb_d = nc.dram_tensor("vb", [64, 16], dt.float32, kind="ExternalInput")
    ot_d = nc.dram_tensor("ot", [D, N1], dt.float32, kind="ExternalOutput")

    with tile.TileContext(nc) as tc, ExitStack() as top:
        # ---- long-lived SBUF tiles ----
        persist = top.enter_context(tc.tile_pool(name="persist", bufs=1))
        qt_sb = persist.tile([128, PAIRS, N1], dt.bfloat16, tag="qt")   # Q^T
        # odd-head halves relocated to partitions 0:64 (operands at SBUF base
        # partition 64 fault the PE on this hardware)
        qt_o = persist.tile([64, PAIRS, N1], dt.bfloat16, tag="qt_o")
        v_sb = persist.tile([128, 512, 65], dt.bfloat16, tag="v")       # [V|1]
        ut_sb = persist.tile([128, PAIRS, N1], dt.bfloat16, tag="ut")   # U^T
        ot_sb = persist.tile([128, 8, N1], dt.float32, tag="ot")        # O^T
        qb_sb = persist.tile([128, 8], dt.float32, tag="qb")
        kb_sb = persist.tile([128, 8], dt.float32, tag="kb")
        pb_sb = persist.tile([128, 8], dt.float32, tag="pb")
        vb_sb = persist.tile([64, 16], dt.float32, tag="vb")
        ones_bf = persist.tile([128, 1], dt.bfloat16, tag="ones_bf")
        ones_f0 = persist.tile([1, 128], dt.float32, tag="ones_f0")

        nc.sync.dma_start(qb_sb[:], qb_d[:])
        nc.sync.dma_start(kb_sb[:], kb_d[:])
        nc.sync.dma_start(pb_sb[:], pb_d[:])
        nc.sync.dma_start(vb_sb[:], vb_d[:])
        if phase < 5:
            nc.vector.memset(ut_sb[:], 0.0)
            nc.vector.memset(ot_sb[:], 0.0)
        nc.vector.memset(ones_bf[:], 1.0)
        nc.vector.memset(ones_f0[:], 1.0)
        # ones column of [V|1]: softmax row-sum lands on PSUM partition 64
        nc.vector.memset(v_sb[:, :, 64:65], 1.0)

        with ExitStack() as proj:
            wpool = proj.enter_context(tc.tile_pool(name="wpool", bufs=1))
            itp = proj.enter_context(tc.tile_pool(name="itp", bufs=1))
            pps = proj.enter_context(
                tc.tile_pool(name="pps", bufs=2, space="PSUM"))

            kw_sb = wpool.tile([128, 8, D], dt.bfloat16, tag="kw")
            it_sb = itp.tile([128, 8, N2], dt.bfloat16, tag="it")

            def emit_it_chunk(ch):
                for c in range(8):
                    nc.sync.dma_start(
                        it_sb[:, c, ch * 1024:(ch + 1) * 1024],
                        it_d[c * 128:(c + 1) * 128, ch * 1024:(ch + 1) * 1024])

            with ExitStack() as qscope:
                qep = qscope.enter_context(tc.tile_pool(name="qep", bufs=1))
                et_sb = qep.tile([128, 8, N1], dt.bfloat16, tag="et")
                qw_sb = qep.tile([128, 8, D], dt.bfloat16, tag="qw")
                for c in range(8):
                    nc.sync.dma_start(et_sb[:, c, :],
                                      et_d[c * 128:(c + 1) * 128, :])
                    nc.sync.dma_start(qw_sb[:, c, :],
                                      qw_d[c * 128:(c + 1) * 128, :])
                emit_it_chunk(0)
                # ---- Q^T projection ----
                for j in range(PAIRS):
                    ps = pps.tile([128, 512], dt.float32, tag="qk_ps")
                    for c in range(8):
                        nc.tensor.matmul(
                            ps[:, 0:N1], qw_sb[:, c, j * 128:(j + 1) * 128],
                            et_sb[:, c, :], start=(c == 0), stop=(c == 7))
                    nc.vector.tensor_scalar_add(qt_sb[:, j, :], ps[:, 0:N1],
                                                qb_sb[:, j:j + 1])
                    nc.sync.dma_start(qt_o[:, j, :], qt_sb[64:128, j, :])

            # ---- V projection (natural layout, strided into [V|1] slots) ----
            with ExitStack() as vscope:
                vwp = vscope.enter_context(tc.tile_pool(name="vwp", bufs=1))
                vw_sb = vwp.tile([128, 8, D], dt.bfloat16, tag="vw")
                for c in range(8):
                    nc.sync.dma_start(vw_sb[:, c, :],
                                      vw_d[c * 128:(c + 1) * 128, :])
                for ch in range(1, 4):
                    emit_it_chunk(ch)
                vps = vscope.enter_context(
                    tc.tile_pool(name="vps", bufs=2, space="PSUM"))
                for t in range(32):
                    for s in range(2):
                        ps = vps.tile([128, 512], dt.float32, tag="v_ps")
                        for c in range(8):
                            nc.tensor.matmul(
                                ps[:], it_sb[:, c, t * 128:(t + 1) * 128],
                                vw_sb[:, c, s * 512:(s + 1) * 512],
                                start=(c == 0), stop=(c == 7))
                        dst = v_sb[:, t * 16 + s * 8: t * 16 + s * 8 + 8, 0:64]
                        nc.vector.tensor_copy(dst, ps[:].rearrange(
                            "p (h d) -> p h d", d=64))

            for c in range(8):
                nc.sync.dma_start(kw_sb[:, c, :], kw_d[c * 128:(c + 1) * 128, :])

            # ---- K^T projection interleaved with attention ----
            ktp = proj.enter_context(tc.tile_pool(name="ktp", bufs=2))
            ktop = proj.enter_context(tc.tile_pool(name="ktop", bufs=2))
            sps = proj.enter_context(
                tc.tile_pool(name="sps", bufs=2, space="PSUM"))
            avp = proj.enter_context(
                tc.tile_pool(name="avp", bufs=1, space="PSUM"))
            ptp = proj.enter_context(tc.tile_pool(name="ptp", bufs=2))
            invp = proj.enter_context(tc.tile_pool(name="invp", bufs=1))
            bsbp = proj.enter_context(tc.tile_pool(name="bsbp", bufs=1))
            stp = proj.enter_context(tc.tile_pool(name="stp", bufs=1))

            kt_tiles = []

            def emit_k_block(j, g8, kt, kt_o):
                ps = pps.tile([128, 512], dt.float32, tag="qk_ps")
                for c in range(8):
                    nc.tensor.matmul(
                        ps[:], kw_sb[:, c, j * 128:(j + 1) * 128],
                        it_sb[:, c, g8 * 512:(g8 + 1) * 512],
                        start=(c == 0), stop=(c == 7))
                nc.vector.tensor_scalar_add(
                    kt[:, g8 * 512:(g8 + 1) * 512], ps[:], kb_sb[:, j:j + 1])
                nc.sync.dma_start(kt_o[:, g8 * 512:(g8 + 1) * 512],
                                  kt[64:128, g8 * 512:(g8 + 1) * 512])

            def emit_attn_groups(p, av_ab, kt, kt_o, groups):
                """groups: iterable of g (0..15); each covers m2 tiles 2g, 2g+1."""
                for g in groups:
                    s_ps = sps.tile([128, 1024], dt.float32, tag="s_ps")
                    for u in range(2):
                        t = 2 * g + u
                        nc.tensor.matmul(
                            s_ps[:, u * 512: u * 512 + 256],
                            kt[0:64, t * 128:(t + 1) * 128],
                            qt_sb[0:64, p, :], start=True, stop=True)
                        nc.tensor.matmul(
                            s_ps[:, u * 512 + 256: u * 512 + 512],
                            kt_o[:, t * 128:(t + 1) * 128],
                            qt_o[:, p, :], start=True, stop=True)
                    pt = ptp.tile([128, 1024], dt.bfloat16, tag="pt")
                    nc.scalar.activation(pt[:], s_ps[:], Exp)
                    if phase == 2 and sub == "a":
                        continue  # S^T + exp only
                    for u in range(2):
                        t = 2 * g + u
                        for a in range(2):
                            h = 2 * p + a
                            nc.tensor.matmul(
                                av_ab[a][0:65, :],
                                v_sb[:, t * 16 + h, :],
                                pt[:, (u * 2 + a) * 256:(u * 2 + a + 1) * 256],
                                start=(t == 0), stop=(t == 31))

            def emit_pair_finalize(p, av_ab):
                if phase == 2:
                    if sub == "a":
                        return
                    # dump accumulators without the broadcast-divide machinery
                    nc.vector.tensor_copy(ut_sb[0:64, p, :], av_ab[0][0:64, :])
                    st2 = stp.tile([64, N1], dt.bfloat16, tag="st")
                    nc.vector.tensor_copy(st2[:], av_ab[1][0:64, :])
                    nc.sync.dma_start(ut_sb[64:128, p, :], st2[:])
                    return
                # evict undivided U^T halves + denominators first so the AV
                # PSUM banks free before the broadcast-divide chain runs
                ue = bsbp.tile([64, 512], dt.bfloat16, tag="ue")
                nc.vector.tensor_copy(ue[:, 0:256], av_ab[0][0:64, :])
                nc.vector.tensor_copy(ue[:, 256:512], av_ab[1][0:64, :])
                # row 64 of each AV accumulator is the softmax denominator;
                # reciprocal on partition 64, then shift the row to partition 0
                inv = invp.tile([65, 512], dt.float32, tag="inv")
                nc.vector.reciprocal(inv[64:65, 0:256], av_ab[0][64:65, :])
                nc.vector.reciprocal(inv[64:65, 256:512], av_ab[1][64:65, :])
                nc.sync.dma_start(inv[0:1, :], inv[64:65, :])
                bc_ps = pps.tile([128, 512], dt.float32, tag="qk_ps")
                nc.tensor.matmul(bc_ps[:], ones_f0[:], inv[0:1, :],
                                 start=True, stop=True)
                bc_sb = bsbp.tile([64, 512], dt.float32, tag="bc_sb")
                nc.vector.tensor_copy(bc_sb[:], bc_ps[0:64, :])
                # even head: divide + v_b straight into rows 0:64 of U^T
                nc.vector.tensor_mul(ut_sb[0:64, p, :], ue[:, 0:256],
                                     bc_sb[:, 0:256])
                nc.vector.tensor_scalar_add(
                    ut_sb[0:64, p, :], ut_sb[0:64, p, :],
                    vb_sb[:, 2 * p:2 * p + 1])
                # odd head: staging, then partition-shift DMA to rows 64:128
                st = stp.tile([64, N1], dt.bfloat16, tag="st")
                nc.vector.tensor_mul(st[:], ue[:, 256:512],
                                     bc_sb[:, 256:512])
                nc.vector.tensor_scalar_add(st[:], st[:],
                                            vb_sb[:, 2 * p + 1:2 * p + 2])
                nc.sync.dma_start(ut_sb[64:128, p, :], st[:])

            prev = None  # (pair_idx, (av_a, av_b), kt, kt_o)
            for j in range(PAIRS):
                kt = ktp.tile([128, N2], dt.bfloat16, tag="kt")
                kt_o = ktop.tile([64, N2], dt.bfloat16, tag="kt_o")
                kt_tiles.append(kt)
                for g8 in range(8):
                    emit_k_block(j, g8, kt, kt_o)
                    if prev is not None and phase >= 2:
                        emit_attn_groups(prev[0], prev[1], prev[2], prev[3],
                                         [2 * g8, 2 * g8 + 1])
                if prev is not None and phase >= 2:
                    emit_pair_finalize(prev[0], prev[1])
                av_a = avp.tile([128, N1], dt.float32, tag="av_a")
                av_b = avp.tile([128, N1], dt.float32, tag="av_b")
                av_ab = (av_a, av_b)
                prev = (j, av_ab, kt, kt_o)
            if phase >= 2:
                emit_attn_groups(prev[0], prev[1], prev[2], prev[3], range(16))
                emit_pair_finalize(prev[0], prev[1])
            if phase == 1:
                # keep K^T tiles alive / observable: dump slices into O^T
                for co in range(8):
                    nc.vector.tensor_copy(ot_sb[:, co, 0:N1],
                                          kt_tiles[co][:, 0:N1])

        # ---- P projection + L2 normalize (projection pools freed) ----
        with ExitStack() as tail:
            pwp = tail.enter_context(tc.tile_pool(name="pwp", bufs=1))
            ops = tail.enter_context(
                tc.tile_pool(name="ops", bufs=2, space="PSUM"))
            nps = tail.enter_context(
                tc.tile_pool(name="nps", bufs=1, space="PSUM"))
            sqp = tail.enter_context(tc.tile_pool(name="sqp", bufs=2))
            fop = tail.enter_context(tc.tile_pool(name="fop", bufs=2))

            pw_sb = pwp.tile([128, 8, D], dt.bfloat16, tag="pw")
            for c in range(8):
                nc.sync.dma_start(pw_sb[:, c, :],
                                  pw_d[c * 128:(c + 1) * 128, :])

            if phase >= 4:
                for co in range(8):
                    ps = ops.tile([128, N1], dt.float32, tag="o_ps")
                    for ci in range(8):
                        nc.tensor.matmul(
                            ps[:], pw_sb[:, ci, co * 128:(co + 1) * 128],
                            ut_sb[:, ci, :], start=(ci == 0), stop=(ci == 7))
                    nc.vector.tensor_scalar_add(ot_sb[:, co, :], ps[:],
                                                pb_sb[:, co:co + 1])

            if phase >= 5:
                nsq = nps.tile([128, N1], dt.float32, tag="nsq")
                for co in range(8):
                    sq = sqp.tile([128, N1], dt.bfloat16, tag="sq")
                    nc.vector.tensor_mul(sq[:], ot_sb[:, co, :],
                                         ot_sb[:, co, :])
                    nc.tensor.matmul(nsq[0:1, :], ones_bf[:], sq[:],
                                     start=(co == 0), stop=(co == 7))
                lnt = sqp.tile([1, N1], dt.float32, tag="lnt")
                nc.scalar.activation(lnt[:], nsq[0:1, :], Ln)
                invn = sqp.tile([1, N1], dt.float32, tag="invn")
                nc.scalar.activation(invn[:], lnt[:], Exp, scale=-0.5)
                bcn = nps.tile([128, N1], dt.float32, tag="bcn")
                nc.tensor.matmul(bcn[:], ones_f0[:], invn[:],
                                 start=True, stop=True)
                for co in range(8):
                    fo = fop.tile([128, N1], dt.float32, tag="fo")
                    nc.vector.tensor_mul(fo[:], ot_sb[:, co, :], bcn[:])
                    nc.sync.dma_start(ot_d[co * 128:(co + 1) * 128, :], fo[:])
            else:
                for co in range(8):
                    fo = fop.tile([128, N1], dt.float32, tag="fo")
                    nc.vector.tensor_copy(fo[:], ot_sb[:, co, :])
                    nc.sync.dma_start(ot_d[co * 128:(co + 1) * 128, :], fo[:])

    nc.compile()
    return nc


def kernel(E, I, q_w, q_b, k_w, k_b, v_w, v_b, p_w, p_b):
    global _COMPILED, LAST_RESULT
    from concourse import bass_utils

    if _COMPILED is None:
        _COMPILED = _build()
    nc = _COMPILED

    E = np.asarray(E, dtype=np.float32)
    I = np.asarray(I, dtype=np.float32)

    def _wT(w):
        return np.ascontiguousarray(np.asarray(w, np.float32).T).astype(BF16)

    qw, kw, vw, pw = _wT(q_w), _wT(k_w), _wT(v_w), _wT(p_w)
    qb = np.ascontiguousarray(np.asarray(q_b, np.float32).reshape(8, 128).T)
    kb = np.ascontiguousarray(np.asarray(k_b, np.float32).reshape(8, 128).T)
    pb = np.ascontiguousarray(np.asarray(p_b, np.float32).reshape(8, 128).T)
    vb = np.ascontiguousarray(np.asarray(v_b, np.float32).reshape(16, 64).T)

    in_maps = []
    for b in range(B):
        in_maps.append({
            "it": np.ascontiguousarray(I[b].T).astype(BF16),
            "et": np.ascontiguousarray(E[b].T).astype(BF16),
            "qw": qw, "kw": kw, "vw": vw, "pw": pw,
            "qb": qb, "kb": kb, "pb": pb, "vb": vb,
        })

    res = bass_utils.run_bass_kernel_spmd(
        nc, in_maps, core_ids=list(range(N_CORES)),
        trace=bool(os.environ.get("BASS_TRACE")))
    LAST_RESULT = res

    out = np.empty((B, N1, 2048), dtype=np.float32)
    for b in range(B):
        out[b, :, :1024] = E[b]
        out[b, :, 1024:] = res.results[b]["ot"].T
    return out



# revision 2
# speedup vs baseline: 1.5848x; 1.5848x over previous
"""Trainium2 Bass kernel for nn_ContextEmbedding (cross-attention context embedding).

Reference math (per batch b):
    Q = E @ q_w.T        [256, 1024]
    K = I @ k_w.T        [4096, 1024]
    V = I @ v_w.T        [4096, 1024]
    S_h = Q_h @ K_h.T    per head (16 heads, head_dim 64)
    P = softmax(S, -1)
    U_h = P_h @ V_h
    O = (U @ p_w.T);  O /= ||O||_2(row)
    out = concat([E, O], -1)   [256, 2048]

Sharding: pure data-parallel over batch B=8 across the 8 NeuronCores (one
batch per core, no collectives). Host pre-transposes/casts and re-assembles
the output (E-passthrough concat happens on host).

The K/V/P projections run in fp8e4m3 with DoubleRow perf mode (contraction
256 per matmul step, 2 fp8 MACs per PE cell per cycle). fp8 weights are
pre-scaled x16 on the host to clear the e4m3 subnormal band; the scale is
compensated entirely host-side: q_w /16 (so QK^T logits come out exact),
v_b/k_b x16, p_b x256, and the trailing L2 normalization cancels the x256
output scale identically. Q projection and the attention core (QK^T,
softmax, AV) stay bf16. U^T is stored fp8 (feeds the fp8 P projection).

Per-core dataflow (all matmuls accumulate in f32 PSUM):
  Q^T [1024,256] and K^T [1024,4096] in o-on-partitions layout, so partition
  tile j holds head pair (2j, 2j+1) in rows 0:64 / 64:128 -> QK^T runs as
  row-group-tiled matmuls producing S^T [m2, n1]. exp() on ScalarE
  (PSUM->SBUF, 1024-wide ops). V in natural [m2, o] layout with a ones column
  appended per head (65-wide stationary) so AV yields U^T rows 0:64 plus the
  softmax row-sum in row 64. Division by the row-sum uses a ones-matmul
  partition broadcast. P-projection consumes U^T directly; the final L2 norm
  reduces over partitions with a ones-matmul and applies 1/sqrt via
  exp(-0.5*ln(x)) on ScalarE.
"""

import os

import numpy as np
import ml_dtypes

B, N1, N2, D = 8, 256, 4096, 1024
H, HD = 16, 64
PAIRS = H // 2  # 8 partition-tiles of head pairs
N_CORES = 8

BF16 = ml_dtypes.bfloat16
F8 = ml_dtypes.float8_e4m3  # == mybir.dt.float8e4
WS = 16.0  # fp8 weight pre-scale (keeps w*16 ~ N(0, 0.32^2) out of subnormals)

_COMPILED = None  # (nc,) cache so repeated kernel() calls skip the rebuild
LAST_RESULT = None  # BassKernelResults of the most recent run (for harnesses)


def _build():
    import concourse.bacc as bacc
    import concourse.mybir as mybir
    from concourse import tile
    from contextlib import ExitStack

    dt = mybir.dt
    Exp = mybir.ActivationFunctionType.Exp
    Ln = mybir.ActivationFunctionType.Ln
    DR = mybir.MatmulPerfMode.DoubleRow

    nc = bacc.Bacc("TRN2", target_bir_lowering=False, debug=False,
                   num_devices=N_CORES)

    # ---- per-core DRAM tensors (host pre-transposed / pre-cast) ----
    it8_d = nc.dram_tensor("it8", [128, 4, 2, N2], dt.float8e4,
                           kind="ExternalInput")
    et_d = nc.dram_tensor("et", [D, N1], dt.bfloat16, kind="ExternalInput")
    qw_d = nc.dram_tensor("qw", [128, 8, 8, 128], dt.bfloat16,
                          kind="ExternalInput")
    kw8_d = nc.dram_tensor("kw8", [128, 4, 2, D], dt.float8e4,
                           kind="ExternalInput")
    vw8_d = nc.dram_tensor("vw8", [128, 4, 2, D], dt.float8e4,
                           kind="ExternalInput")
    pw8_d = nc.dram_tensor("pw8", [128, 4, 2, D], dt.float8e4,
                           kind="ExternalInput")
    qb_d = nc.dram_tensor("qb", [128, 8], dt.float32, kind="ExternalInput")
    kb_d = nc.dram_tensor("kb", [128, 8], dt.float32, kind="ExternalInput")
    pb_d = nc.dram_tensor("pb", [128, 8], dt.float32, kind="ExternalInput")
    vb_d = nc.dram_tensor("vb", [64, 16], dt.float32, kind="ExternalInput")
    ot_d = nc.dram_tensor("ot", [D, N1], dt.float32, kind="ExternalOutput")

    with tile.TileContext(nc) as tc, ExitStack() as top:
        # ---- long-lived SBUF tiles ----
        persist = top.enter_context(tc.tile_pool(name="persist", bufs=1))
        qt_sb = persist.tile([128, PAIRS, N1], dt.bfloat16, tag="qt")   # Q^T
        # odd-head halves relocated to partitions 0:64 (operands at SBUF base
        # partition 64 fault the PE on this hardware)
        qt_o = persist.tile([64, PAIRS, N1], dt.bfloat16, tag="qt_o")
        v_sb = persist.tile([128, 512, 65], dt.bfloat16, tag="v")       # [V|1]
        ut_sb = persist.tile([128, PAIRS, N1], dt.float8e4, tag="ut")   # U^T
        ot_sb = persist.tile([128, 8, N1], dt.float32, tag="ot")        # O^T
        qb_sb = persist.tile([128, 8], dt.float32, tag="qb")
        kb_sb = persist.tile([128, 8], dt.float32, tag="kb")
        pb_sb = persist.tile([128, 8], dt.float32, tag="pb")
        vb_sb = persist.tile([64, 16], dt.float32, tag="vb")
        ones_bf = persist.tile([128, 1], dt.bfloat16, tag="ones_bf")
        ones_f0 = persist.tile([1, 128], dt.float32, tag="ones_f0")
        # fp8 operands stay resident for the whole kernel
        it8_sb = persist.tile([128, 4, 2, N2], dt.float8e4, tag="it8")
        kw8_sb = persist.tile([128, 4, 2, D], dt.float8e4, tag="kw8")
        vw8_sb = persist.tile([128, 4, 2, D], dt.float8e4, tag="vw8")
        pw8_sb = persist.tile([128, 4, 2, D], dt.float8e4, tag="pw8")

        nc.sync.dma_start(qb_sb[:], qb_d[:])
        nc.sync.dma_start(kb_sb[:], kb_d[:])
        nc.sync.dma_start(pb_sb[:], pb_d[:])
        nc.sync.dma_start(vb_sb[:], vb_d[:])
        nc.vector.memset(ones_bf[:], 1.0)
        nc.vector.memset(ones_f0[:], 1.0)
        # ones column of [V|1]: softmax row-sum lands on PSUM partition 64
        nc.vector.memset(v_sb[:, :, 64:65], 1.0)

        def emit_it_chunk(ch):
            for c in range(4):
                nc.sync.dma_start(
                    it8_sb[:, c, :, ch * 1024:(ch + 1) * 1024],
                    it8_d[:, c, :, ch * 1024:(ch + 1) * 1024])

        with ExitStack() as proj:
            pps = proj.enter_context(
                tc.tile_pool(name="pps", bufs=2, space="PSUM"))

            with ExitStack() as qscope:
                qep = qscope.enter_context(tc.tile_pool(name="qep", bufs=1))
                et_sb = qep.tile([128, 8, N1], dt.bfloat16, tag="et")
                qw_sb = qep.tile([128, 8, 8, 128], dt.bfloat16, tag="qw")
                for c in range(8):
                    nc.sync.dma_start(et_sb[:, c, :],
                                      et_d[c * 128:(c + 1) * 128, :])
                for j in range(PAIRS):
                    nc.sync.dma_start(qw_sb[:, j, :, :], qw_d[:, j, :, :])
                nc.sync.dma_start(vw8_sb[:], vw8_d[:])
                emit_it_chunk(0)
                nc.sync.dma_start(kw8_sb[:], kw8_d[:])
                # ---- Q^T projection (bf16) ----
                for j in range(PAIRS):
                    ps = pps.tile([128, 512], dt.float32, tag="qk_ps")
                    for c in range(8):
                        nc.tensor.matmul(
                            ps[:, 0:N1], qw_sb[:, j, c, :],
                            et_sb[:, c, :], start=(c == 0), stop=(c == 7))
                    nc.vector.tensor_scalar_add(qt_sb[:, j, :], ps[:, 0:N1],
                                                qb_sb[:, j:j + 1])
                    nc.sync.dma_start(qt_o[:, j, :], qt_sb[64:128, j, :])

            # ---- V projection (fp8 DoubleRow, natural layout) ----
            with ExitStack() as vscope:
                for ch in range(1, 4):
                    emit_it_chunk(ch)
                nc.sync.dma_start(pw8_sb[:], pw8_d[:])
                vps = vscope.enter_context(
                    tc.tile_pool(name="vps", bufs=2, space="PSUM"))
                for t in range(32):
                    for s in range(2):
                        ps = vps.tile([128, 512], dt.float32, tag="v_ps")
                        for c in range(4):
                            nc.tensor.matmul(
                                ps[:], it8_sb[:, c, :, t * 128:(t + 1) * 128],
                                vw8_sb[:, c, :, s * 512:(s + 1) * 512],
                                perf_mode=DR, start=(c == 0), stop=(c == 3))
                        dst = v_sb[:, t * 16 + s * 8: t * 16 + s * 8 + 8, 0:64]
                        nc.vector.tensor_copy(dst, ps[:].rearrange(
                            "p (h d) -> p h d", d=64))

            # ---- K^T projection (fp8 DoubleRow) interleaved with attention ----
            ktp = proj.enter_context(tc.tile_pool(name="ktp", bufs=2))
            ktop = proj.enter_context(tc.tile_pool(name="ktop", bufs=2))
            sps = proj.enter_context(
                tc.tile_pool(name="sps", bufs=2, space="PSUM"))
            avp = proj.enter_context(
                tc.tile_pool(name="avp", bufs=1, space="PSUM"))
            ptp = proj.enter_context(tc.tile_pool(name="ptp", bufs=2))
            invp = proj.enter_context(tc.tile_pool(name="invp", bufs=1))
            bsbp = proj.enter_context(tc.tile_pool(name="bsbp", bufs=1))
            stp = proj.enter_context(tc.tile_pool(name="stp", bufs=1))

            def emit_k_block(j, g8, kt, kt_o):
                ps = pps.tile([128, 512], dt.float32, tag="qk_ps")
                for c in range(4):
                    nc.tensor.matmul(
                        ps[:], kw8_sb[:, c, :, j * 128:(j + 1) * 128],
                        it8_sb[:, c, :, g8 * 512:(g8 + 1) * 512],
                        perf_mode=DR, start=(c == 0), stop=(c == 3))
                nc.vector.tensor_scalar_add(
                    kt[:, g8 * 512:(g8 + 1) * 512], ps[:], kb_sb[:, j:j + 1])
                nc.sync.dma_start(kt_o[:, g8 * 512:(g8 + 1) * 512],
                                  kt[64:128, g8 * 512:(g8 + 1) * 512])

            def emit_attn_groups(p, av_ab, kt, kt_o, groups):
                """groups: iterable of g (0..15); each covers m2 tiles 2g, 2g+1."""
                for g in groups:
                    s_ps = sps.tile([128, 1024], dt.float32, tag="s_ps")
                    for u in range(2):
                        t = 2 * g + u
                        nc.tensor.matmul(
                            s_ps[:, u * 512: u * 512 + 256],
                            kt[0:64, t * 128:(t + 1) * 128],
                            qt_sb[0:64, p, :], start=True, stop=True)
                        nc.tensor.matmul(
                            s_ps[:, u * 512 + 256: u * 512 + 512],
                            kt_o[:, t * 128:(t + 1) * 128],
                            qt_o[:, p, :], start=True, stop=True)
                    pt = ptp.tile([128, 1024], dt.bfloat16, tag="pt")
                    nc.scalar.activation(pt[:], s_ps[:], Exp)
                    for u in range(2):
                        t = 2 * g + u
                        for a in range(2):
                            h = 2 * p + a
                            nc.tensor.matmul(
                                av_ab[a][0:65, :],
                                v_sb[:, t * 16 + h, :],
                                pt[:, (u * 2 + a) * 256:(u * 2 + a + 1) * 256],
                                start=(t == 0), stop=(t == 31))

            def emit_pair_finalize(p, av_ab):
                # evict undivided U^T halves + denominators first so the AV
                # PSUM banks free before the broadcast-divide chain runs
                ue = bsbp.tile([64, 512], dt.bfloat16, tag="ue")
                nc.vector.tensor_copy(ue[:, 0:256], av_ab[0][0:64, :])
                nc.vector.tensor_copy(ue[:, 256:512], av_ab[1][0:64, :])
                # row 64 of each AV accumulator is the softmax denominator;
                # reciprocal on partition 64, then shift the row to partition 0
                inv = invp.tile([65, 512], dt.float32, tag="inv")
                nc.vector.reciprocal(inv[64:65, 0:256], av_ab[0][64:65, :])
                nc.vector.reciprocal(inv[64:65, 256:512], av_ab[1][64:65, :])
                nc.sync.dma_start(inv[0:1, :], inv[64:65, :])
                bc_ps = pps.tile([128, 512], dt.float32, tag="qk_ps")
                nc.tensor.matmul(bc_ps[:], ones_f0[:], inv[0:1, :],
                                 start=True, stop=True)
                bc_sb = bsbp.tile([64, 512], dt.float32, tag="bc_sb")
                nc.vector.tensor_copy(bc_sb[:], bc_ps[0:64, :])
                # even head: divide + v_b straight into rows 0:64 of U^T
                nc.vector.tensor_mul(ut_sb[0:64, p, :], ue[:, 0:256],
                                     bc_sb[:, 0:256])
                nc.vector.tensor_scalar_add(
                    ut_sb[0:64, p, :], ut_sb[0:64, p, :],
                    vb_sb[:, 2 * p:2 * p + 1])
                # odd head: staging, then partition-shift DMA to rows 64:128
                st = stp.tile([64, N1], dt.float8e4, tag="st")
                nc.vector.tensor_mul(st[:], ue[:, 256:512],
                                     bc_sb[:, 256:512])
                nc.vector.tensor_scalar_add(st[:], st[:],
                                            vb_sb[:, 2 * p + 1:2 * p + 2])
                nc.sync.dma_start(ut_sb[64:128, p, :], st[:])

            prev = None  # (pair_idx, (av_a, av_b), kt, kt_o)
            for j in range(PAIRS):
                kt = ktp.tile([128, N2], dt.bfloat16, tag="kt")
                kt_o = ktop.tile([64, N2], dt.bfloat16, tag="kt_o")
                for g8 in range(8):
                    emit_k_block(j, g8, kt, kt_o)
                    if prev is not None:
                        emit_attn_groups(prev[0], prev[1], prev[2], prev[3],
                                         [2 * g8, 2 * g8 + 1])
                if prev is not None:
                    emit_pair_finalize(prev[0], prev[1])
                av_a = avp.tile([128, N1], dt.float32, tag="av_a")
                av_b = avp.tile([128, N1], dt.float32, tag="av_b")
                av_ab = (av_a, av_b)
                prev = (j, av_ab, kt, kt_o)
            emit_attn_groups(prev[0], prev[1], prev[2], prev[3], range(16))
            emit_pair_finalize(prev[0], prev[1])

        # ---- P projection (fp8 DoubleRow) + L2 normalize ----
        with ExitStack() as tail:
            ops = tail.enter_context(
                tc.tile_pool(name="ops", bufs=2, space="PSUM"))
            nps = tail.enter_context(
                tc.tile_pool(name="nps", bufs=1, space="PSUM"))
            sqp = tail.enter_context(tc.tile_pool(name="sqp", bufs=2))
            fop = tail.enter_context(tc.tile_pool(name="fop", bufs=2))

            for co in range(8):
                ps = ops.tile([128, N1], dt.float32, tag="o_ps")
                for c2 in range(4):
                    nc.tensor.matmul(
                        ps[:], pw8_sb[:, c2, :, co * 128:(co + 1) * 128],
                        ut_sb[:, 2 * c2:2 * c2 + 2, :],
                        perf_mode=DR, start=(c2 == 0), stop=(c2 == 3))
                nc.vector.tensor_scalar_add(ot_sb[:, co, :], ps[:],
                                            pb_sb[:, co:co + 1])

            nsq = nps.tile([128, N1], dt.float32, tag="nsq")
            for co in range(8):
                sq = sqp.tile([128, N1], dt.bfloat16, tag="sq")
                nc.vector.tensor_mul(sq[:], ot_sb[:, co, :],
                                     ot_sb[:, co, :])
                nc.tensor.matmul(nsq[0:1, :], ones_bf[:], sq[:],
                                 start=(co == 0), stop=(co == 7))
            lnt = sqp.tile([1, N1], dt.float32, tag="lnt")
            nc.scalar.activation(lnt[:], nsq[0:1, :], Ln)
            invn = sqp.tile([1, N1], dt.float32, tag="invn")
            nc.scalar.activation(invn[:], lnt[:], Exp, scale=-0.5)
            bcn = nps.tile([128, N1], dt.float32, tag="bcn")
            nc.tensor.matmul(bcn[:], ones_f0[:], invn[:],
                             start=True, stop=True)
            for co in range(8):
                fo = fop.tile([128, N1], dt.float32, tag="fo")
                nc.vector.tensor_mul(fo[:], ot_sb[:, co, :], bcn[:])
                nc.sync.dma_start(ot_d[co * 128:(co + 1) * 128, :], fo[:])

    nc.compile()
    return nc


def _pair8(a):
    """[1024, X] f32 -> [128, 4, 2, X] fp8e4m3 DoubleRow-paired layout."""
    x = a.reshape(4, 2, 128, -1).transpose(2, 0, 1, 3)
    return np.ascontiguousarray(x).astype(F8)


def kernel(E, I, q_w, q_b, k_w, k_b, v_w, v_b, p_w, p_b):
    global _COMPILED, LAST_RESULT
    from concourse import bass_utils

    if _COMPILED is None:
        _COMPILED = _build()
    nc = _COMPILED

    E = np.asarray(E, dtype=np.float32)
    I = np.asarray(I, dtype=np.float32)

    kw8 = _pair8(WS * np.asarray(k_w, np.float32).T)
    vw8 = _pair8(WS * np.asarray(v_w, np.float32).T)
    pw8 = _pair8(WS * np.asarray(p_w, np.float32).T)
    # Q^T weights scaled 1/WS (cancels K's x WS in the logits), laid out as
    # [p, j, c, m] blocks so per-j DMAs are contiguous.
    qw = (np.asarray(q_w, np.float32).T / WS).reshape(8, 128, 8, 128)
    qw = np.ascontiguousarray(qw.transpose(1, 2, 0, 3)).astype(BF16)

    qb = np.ascontiguousarray(
        (np.asarray(q_b, np.float32) / WS).reshape(8, 128).T)
    kb = np.ascontiguousarray(
        (WS * np.asarray(k_b, np.float32)).reshape(8, 128).T)
    pb = np.ascontiguousarray(
        (WS * WS * np.asarray(p_b, np.float32)).reshape(8, 128).T)
    vb = np.ascontiguousarray(
        (WS * np.asarray(v_b, np.float32)).reshape(16, 64).T)

    in_maps = []
    for b in range(B):
        in_maps.append({
            "it8": _pair8(I[b].T),
            "et": np.ascontiguousarray(E[b].T).astype(BF16),
            "qw": qw, "kw8": kw8, "vw8": vw8, "pw8": pw8,
            "qb": qb, "kb": kb, "pb": pb, "vb": vb,
        })

    res = bass_utils.run_bass_kernel_spmd(
        nc, in_maps, core_ids=list(range(N_CORES)),
        trace=bool(os.environ.get("BASS_TRACE")))
    LAST_RESULT = res

    out = np.empty((B, N1, 2048), dtype=np.float32)
    for b in range(B):
        out[b, :, :1024] = E[b]
        out[b, :, 1024:] = res.results[b]["ot"].T
    return out


# revision 12
# speedup vs baseline: 1.6191x; 1.0217x over previous
"""Trainium2 Bass kernel for nn_ContextEmbedding (cross-attention context embedding).

Reference math (per batch b):
    Q = E @ q_w.T        [256, 1024]
    K = I @ k_w.T        [4096, 1024]
    V = I @ v_w.T        [4096, 1024]
    S_h = Q_h @ K_h.T    per head (16 heads, head_dim 64)
    P = softmax(S, -1)
    U_h = P_h @ V_h
    O = (U @ p_w.T);  O /= ||O||_2(row)
    out = concat([E, O], -1)   [256, 2048]

Sharding: pure data-parallel over batch B=8 across the 8 NeuronCores (one
batch per core, no collectives). Host pre-transposes/casts and re-assembles
the output (E-passthrough concat happens on host).

The K/V/P projections run in fp8e4m3 with DoubleRow perf mode (contraction
256 per matmul step, 2 fp8 MACs per PE cell per cycle). fp8 weights are
pre-scaled x16 on the host to clear the e4m3 subnormal band; the scale is
compensated entirely host-side: q_w /16 (so QK^T logits come out exact),
v_b/k_b x16, p_b x256, and the trailing L2 normalization cancels the x256
output scale identically. Q projection and the attention core (QK^T,
softmax, AV) stay bf16. U^T is stored fp8 (feeds the fp8 P projection).

Per-core dataflow (all matmuls accumulate in f32 PSUM):
  Q^T [1024,256] and K^T [1024,4096] in o-on-partitions layout, so partition
  tile j holds head pair (2j, 2j+1) in rows 0:64 / 64:128 -> QK^T runs as
  row-group-tiled matmuls producing S^T [m2, n1]. exp() on ScalarE
  (PSUM->SBUF, 1024-wide ops). V in natural [m2, o] layout with a ones column
  appended per head (65-wide stationary) so AV yields U^T rows 0:64 plus the
  softmax row-sum in row 64. Division by the row-sum uses a ones-matmul
  partition broadcast. P-projection consumes U^T directly; the final L2 norm
  reduces over partitions with a ones-matmul and applies 1/sqrt via
  exp(-0.5*ln(x)) on ScalarE.
"""

import os

import numpy as np
import ml_dtypes

B, N1, N2, D = 8, 256, 4096, 1024
H, HD = 16, 64
PAIRS = H // 2  # 8 partition-tiles of head pairs
N_CORES = 8

BF16 = ml_dtypes.bfloat16
F8 = ml_dtypes.float8_e4m3  # == mybir.dt.float8e4
WS = 16.0  # fp8 weight pre-scale (keeps w*16 ~ N(0, 0.32^2) out of subnormals)

_COMPILED = None  # (nc,) cache so repeated kernel() calls skip the rebuild
LAST_RESULT = None  # BassKernelResults of the most recent run (for harnesses)


def _build():
    import concourse.bacc as bacc
    import concourse.mybir as mybir
    from concourse import tile
    from contextlib import ExitStack

    dt = mybir.dt
    Exp = mybir.ActivationFunctionType.Exp
    Ln = mybir.ActivationFunctionType.Ln
    DR = mybir.MatmulPerfMode.DoubleRow

    nc = bacc.Bacc("TRN2", target_bir_lowering=False, debug=False,
                   num_devices=N_CORES)

    # ---- per-core DRAM tensors (host pre-transposed / pre-cast) ----
    it8_d = nc.dram_tensor("it8", [128, 4, 2, N2], dt.float8e4,
                           kind="ExternalInput")
    et_d = nc.dram_tensor("et", [128, 8, N1], dt.bfloat16,
                          kind="ExternalInput")
    qw_d = nc.dram_tensor("qw", [128, 8, 8, 128], dt.bfloat16,
                          kind="ExternalInput")
    kw8_d = nc.dram_tensor("kw8", [128, 4, 2, D], dt.float8e4,
                           kind="ExternalInput")
    vw8_d = nc.dram_tensor("vw8", [128, 4, 2, D], dt.float8e4,
                           kind="ExternalInput")
    pw8_d = nc.dram_tensor("pw8", [128, 4, 2, D], dt.float8e4,
                           kind="ExternalInput")
    bias_d = nc.dram_tensor("bias", [128, 40], dt.float32,
                            kind="ExternalInput")
    ot_d = nc.dram_tensor("ot", [D, N1], dt.float32, kind="ExternalOutput")

    with tile.TileContext(nc) as tc, ExitStack() as top:
        # ---- long-lived SBUF tiles ----
        persist = top.enter_context(tc.tile_pool(name="persist", bufs=1))
        qt_sb = persist.tile([128, PAIRS, N1], dt.bfloat16, tag="qt")   # Q^T
        v_sb = persist.tile([128, 512, 65], dt.bfloat16, tag="v")       # [V|1]
        ut_sb = persist.tile([128, PAIRS, N1], dt.float8e4, tag="ut")   # U^T
        ot_sb = persist.tile([128, 8, N1], dt.float32, tag="ot")        # O^T
        # packed biases: cols 0:8 qb, 8:16 kb, 16:24 pb, 24:40 vb (rows 0:64)
        bias_sb = persist.tile([128, 40], dt.float32, tag="bias")
        qb_sb = bias_sb[:, 0:8]
        kb_sb = bias_sb[:, 8:16]
        pb_sb = bias_sb[:, 16:24]
        vb_sb = bias_sb[0:64, 24:40]
        ones_bf = persist.tile([128, 1], dt.bfloat16, tag="ones_bf")
        ones_f0 = persist.tile([1, 128], dt.float32, tag="ones_f0")
        # fp8 operands stay resident for the whole kernel
        it8_sb = persist.tile([128, 4, 2, N2], dt.float8e4, tag="it8")
        kw8_sb = persist.tile([128, 4, 2, D], dt.float8e4, tag="kw8")
        vw8_sb = persist.tile([128, 4, 2, D], dt.float8e4, tag="vw8")
        pw8_sb = persist.tile([128, 4, 2, D], dt.float8e4, tag="pw8")

        nc.sync.dma_start(bias_sb[:], bias_d[:])
        nc.vector.memset(ones_bf[:], 1.0)
        nc.vector.memset(ones_f0[:], 1.0)
        # ones column of [V|1]: softmax row-sum lands on PSUM partition 64
        nc.vector.memset(v_sb[:, :, 64:65], 1.0)

        def emit_it_chunk(ch):
            for c in range(4):
                nc.sync.dma_start(
                    it8_sb[:, c, :, ch * 1024:(ch + 1) * 1024],
                    it8_d[:, c, :, ch * 1024:(ch + 1) * 1024])

        with ExitStack() as proj:
            pps = proj.enter_context(
                tc.tile_pool(name="pps", bufs=2, space="PSUM"))

            with ExitStack() as qscope:
                qep = qscope.enter_context(tc.tile_pool(name="qep", bufs=1))
                et_sb = qep.tile([128, 8, N1], dt.bfloat16, tag="et")
                qw_sb = qep.tile([128, 8, 8, 128], dt.bfloat16, tag="qw")
                nc.sync.dma_start(et_sb[:], et_d[:])
                for j in range(PAIRS):
                    nc.sync.dma_start(qw_sb[:, j, :, :], qw_d[:, j, :, :])
                nc.sync.dma_start(vw8_sb[:], vw8_d[:])
                emit_it_chunk(0)
                nc.sync.dma_start(kw8_sb[:], kw8_d[:])
                # ---- Q^T projection (bf16) ----
                for j in range(PAIRS):
                    ps = pps.tile([128, 512], dt.float32, tag="qk_ps")
                    for c in range(8):
                        nc.tensor.matmul(
                            ps[:, 0:N1], qw_sb[:, j, c, :],
                            et_sb[:, c, :], start=(c == 0), stop=(c == 7))
                    nc.vector.tensor_scalar_add(qt_sb[:, j, :], ps[:, 0:N1],
                                                qb_sb[:, j:j + 1])

            # ---- V projection (fp8 DoubleRow, natural layout) ----
            with ExitStack() as vscope:
                for ch in range(1, 4):
                    emit_it_chunk(ch)
                nc.sync.dma_start(pw8_sb[:], pw8_d[:])
                vps = vscope.enter_context(
                    tc.tile_pool(name="vps", bufs=2, space="PSUM"))
                for t in range(32):
                    for s in range(2):
                        ps = vps.tile([128, 512], dt.float32, tag="v_ps")
                        for c in range(4):
                            nc.tensor.matmul(
                                ps[:], it8_sb[:, c, :, t * 128:(t + 1) * 128],
                                vw8_sb[:, c, :, s * 512:(s + 1) * 512],
                                perf_mode=DR, start=(c == 0), stop=(c == 3))
                        dst = v_sb[:, t * 16 + s * 8: t * 16 + s * 8 + 8, 0:64]
                        nc.vector.tensor_copy(dst, ps[:].rearrange(
                            "p (h d) -> p h d", d=64))

            # ---- K^T projection (fp8 DoubleRow) interleaved with attention ----
            ktp = proj.enter_context(tc.tile_pool(name="ktp", bufs=2))
            sps = proj.enter_context(
                tc.tile_pool(name="sps", bufs=2, space="PSUM"))
            avp = proj.enter_context(
                tc.tile_pool(name="avp", bufs=1, space="PSUM"))
            ptp = proj.enter_context(tc.tile_pool(name="ptp", bufs=2))
            invp = proj.enter_context(tc.tile_pool(name="invp", bufs=1))
            bsbp = proj.enter_context(tc.tile_pool(name="bsbp", bufs=1))
            stp = proj.enter_context(tc.tile_pool(name="stp", bufs=1))

            def emit_k_block(j, g8, kt):
                ps = pps.tile([128, 512], dt.float32, tag="qk_ps")
                for c in range(4):
                    nc.tensor.matmul(
                        ps[:], kw8_sb[:, c, :, j * 128:(j + 1) * 128],
                        it8_sb[:, c, :, g8 * 512:(g8 + 1) * 512],
                        perf_mode=DR, start=(c == 0), stop=(c == 3))
                nc.vector.tensor_scalar_add(
                    kt[:, g8 * 512:(g8 + 1) * 512], ps[:], kb_sb[:, j:j + 1])

            def emit_attn_groups(p, av_ab, kt, groups):
                """groups: iterable of g (0..15); each covers m2 tiles 2g, 2g+1.

                Even head runs on PE row-group 0:64, odd head on 64:128
                (tile_position auto-derived from base partition) so the two
                64-deep QK^T matmuls execute concurrently. Their outputs land
                in different PSUM banks (cols 0:512 vs 512:1024) to keep the
                concurrent drains conflict-free.
                """
                for g in groups:
                    s_ps = sps.tile([128, 1024], dt.float32, tag="s_ps")
                    for u in range(2):
                        t = 2 * g + u
                        nc.tensor.matmul(
                            s_ps[:, u * 256:(u + 1) * 256],
                            kt[0:64, t * 128:(t + 1) * 128],
                            qt_sb[0:64, p, :], start=True, stop=True)
                        nc.tensor.matmul(
                            s_ps[:, 512 + u * 256: 512 + (u + 1) * 256],
                            kt[64:128, t * 128:(t + 1) * 128],
                            qt_sb[64:128, p, :], start=True, stop=True)
                    pt = ptp.tile([128, 1024], dt.bfloat16, tag="pt")
                    nc.scalar.activation(pt[:], s_ps[:], Exp)
                    for u in range(2):
                        t = 2 * g + u
                        for a in range(2):
                            h = 2 * p + a
                            nc.tensor.matmul(
                                av_ab[a][0:65, :],
                                v_sb[:, t * 16 + h, :],
                                pt[:, (a * 2 + u) * 256:(a * 2 + u + 1) * 256],
                                start=(t == 0), stop=(t == 31))

            def emit_pair_finalize(p, av_ab):
                # evict undivided U^T halves + denominators first so the AV
                # PSUM banks free before the broadcast-divide chain runs
                ue = bsbp.tile([64, 512], dt.bfloat16, tag="ue")
                nc.vector.tensor_copy(ue[:, 0:256], av_ab[0][0:64, :])
                nc.vector.tensor_copy(ue[:, 256:512], av_ab[1][0:64, :])
                # row 64 of each AV accumulator is the softmax denominator;
                # reciprocal on partition 64, then shift the row to partition 0
                inv = invp.tile([65, 512], dt.float32, tag="inv")
                nc.vector.reciprocal(inv[64:65, 0:256], av_ab[0][64:65, :])
                nc.vector.reciprocal(inv[64:65, 256:512], av_ab[1][64:65, :])
                nc.sync.dma_start(inv[0:1, :], inv[64:65, :])
                bc_ps = pps.tile([128, 512], dt.float32, tag="qk_ps")
                nc.tensor.matmul(bc_ps[:], ones_f0[:], inv[0:1, :],
                                 start=True, stop=True)
                bc_sb = bsbp.tile([64, 512], dt.float32, tag="bc_sb")
                nc.vector.tensor_copy(bc_sb[:], bc_ps[0:64, :])
                # even head: divide + v_b straight into rows 0:64 of U^T
                nc.vector.tensor_mul(ut_sb[0:64, p, :], ue[:, 0:256],
                                     bc_sb[:, 0:256])
                nc.vector.tensor_scalar_add(
                    ut_sb[0:64, p, :], ut_sb[0:64, p, :],
                    vb_sb[:, 2 * p:2 * p + 1])
                # odd head: staging, then partition-shift DMA to rows 64:128
                st = stp.tile([64, N1], dt.float8e4, tag="st")
                nc.vector.tensor_mul(st[:], ue[:, 256:512],
                                     bc_sb[:, 256:512])
                nc.vector.tensor_scalar_add(st[:], st[:],
                                            vb_sb[:, 2 * p + 1:2 * p + 2])
                nc.sync.dma_start(ut_sb[64:128, p, :], st[:])

            prev = None  # (pair_idx, (av_a, av_b), kt)
            for j in range(PAIRS):
                kt = ktp.tile([128, N2], dt.bfloat16, tag="kt")
                for g8 in range(8):
                    emit_k_block(j, g8, kt)
                    if prev is not None:
                        emit_attn_groups(prev[0], prev[1], prev[2],
                                         [2 * g8, 2 * g8 + 1])
                if prev is not None:
                    emit_pair_finalize(prev[0], prev[1])
                av_a = avp.tile([128, N1], dt.float32, tag="av_a")
                av_b = avp.tile([128, N1], dt.float32, tag="av_b")
                av_ab = (av_a, av_b)
                prev = (j, av_ab, kt)
            emit_attn_groups(prev[0], prev[1], prev[2], range(16))
            emit_pair_finalize(prev[0], prev[1])

        # ---- P projection (fp8 DoubleRow) + L2 normalize ----
        with ExitStack() as tail:
            ops = tail.enter_context(
                tc.tile_pool(name="ops", bufs=2, space="PSUM"))
            nps = tail.enter_context(
                tc.tile_pool(name="nps", bufs=1, space="PSUM"))
            sqp = tail.enter_context(tc.tile_pool(name="sqp", bufs=2))
            fop = tail.enter_context(tc.tile_pool(name="fop", bufs=2))

            for co in range(8):
                ps = ops.tile([128, N1], dt.float32, tag="o_ps")
                for c2 in range(4):
                    nc.tensor.matmul(
                        ps[:], pw8_sb[:, c2, :, co * 128:(co + 1) * 128],
                        ut_sb[:, 2 * c2:2 * c2 + 2, :],
                        perf_mode=DR, start=(c2 == 0), stop=(c2 == 3))
                nc.vector.tensor_scalar_add(ot_sb[:, co, :], ps[:],
                                            pb_sb[:, co:co + 1])

            nsq = nps.tile([128, N1], dt.float32, tag="nsq")
            for co in range(8):
                sq = sqp.tile([128, N1], dt.bfloat16, tag="sq")
                nc.vector.tensor_mul(sq[:], ot_sb[:, co, :],
                                     ot_sb[:, co, :])
                nc.tensor.matmul(nsq[0:1, :], ones_bf[:], sq[:],
                                 start=(co == 0), stop=(co == 7))
            lnt = sqp.tile([1, N1], dt.float32, tag="lnt")
            nc.scalar.activation(lnt[:], nsq[0:1, :], Ln)
            invn = sqp.tile([1, N1], dt.float32, tag="invn")
            nc.scalar.activation(invn[:], lnt[:], Exp, scale=-0.5)
            bcn = nps.tile([128, N1], dt.float32, tag="bcn")
            nc.tensor.matmul(bcn[:], ones_f0[:], invn[:],
                             start=True, stop=True)
            for co in range(8):
                fo = fop.tile([128, N1], dt.float32, tag="fo")
                nc.vector.tensor_mul(fo[:], ot_sb[:, co, :], bcn[:])
                nc.sync.dma_start(ot_d[co * 128:(co + 1) * 128, :], fo[:])

    nc.compile()
    return nc


def _pair8(a):
    """[1024, X] f32 -> [128, 4, 2, X] fp8e4m3 DoubleRow-paired layout."""
    x = a.reshape(4, 2, 128, -1).transpose(2, 0, 1, 3)
    return np.ascontiguousarray(x).astype(F8)


def kernel(E, I, q_w, q_b, k_w, k_b, v_w, v_b, p_w, p_b):
    global _COMPILED, LAST_RESULT
    from concourse import bass_utils

    if _COMPILED is None:
        _COMPILED = _build()
    nc = _COMPILED

    E = np.asarray(E, dtype=np.float32)
    I = np.asarray(I, dtype=np.float32)

    kw8 = _pair8(WS * np.asarray(k_w, np.float32).T)
    vw8 = _pair8(WS * np.asarray(v_w, np.float32).T)
    pw8 = _pair8(WS * np.asarray(p_w, np.float32).T)
    # Q^T weights scaled 1/WS (cancels K's x WS in the logits), laid out as
    # [p, j, c, m] blocks so per-j DMAs are contiguous.
    qw = (np.asarray(q_w, np.float32).T / WS).reshape(8, 128, 8, 128)
    qw = np.ascontiguousarray(qw.transpose(1, 2, 0, 3)).astype(BF16)

    bias = np.zeros((128, 40), np.float32)
    bias[:, 0:8] = (np.asarray(q_b, np.float32) / WS).reshape(8, 128).T
    bias[:, 8:16] = (WS * np.asarray(k_b, np.float32)).reshape(8, 128).T
    bias[:, 16:24] = (WS * WS * np.asarray(p_b, np.float32)).reshape(8, 128).T
    bias[0:64, 24:40] = (WS * np.asarray(v_b, np.float32)).reshape(16, 64).T

    in_maps = []
    for b in range(B):
        et = E[b].T.reshape(8, 128, N1).transpose(1, 0, 2)
        in_maps.append({
            "it8": _pair8(I[b].T),
            "et": np.ascontiguousarray(et).astype(BF16),
            "qw": qw, "kw8": kw8, "vw8": vw8, "pw8": pw8,
            "bias": bias,
        })

    res = bass_utils.run_bass_kernel_spmd(
        nc, in_maps, core_ids=list(range(N_CORES)),
        trace=bool(os.environ.get("BASS_TRACE")))
    LAST_RESULT = res

    out = np.empty((B, N1, 2048), dtype=np.float32)
    for b in range(B):
        out[b, :, :1024] = E[b]
        out[b, :, 1024:] = res.results[b]["ot"].T
    return out


# revision 16
# speedup vs baseline: 1.7359x; 1.0721x over previous
"""Trainium2 Bass kernel for nn_ContextEmbedding (cross-attention context embedding).

Reference math (per batch b):
    Q = E @ q_w.T        [256, 1024]
    K = I @ k_w.T        [4096, 1024]
    V = I @ v_w.T        [4096, 1024]
    S_h = Q_h @ K_h.T    per head (16 heads, head_dim 64)
    P = softmax(S, -1)
    U_h = P_h @ V_h
    O = (U @ p_w.T);  O /= ||O||_2(row)
    out = concat([E, O], -1)   [256, 2048]

Sharding: pure data-parallel over batch B=8 across the 8 NeuronCores (one
batch per core, no collectives). Host pre-transposes/casts and re-assembles
the output (E-passthrough concat happens on host).

The K/V/P projections run in fp8e4m3 with DoubleRow perf mode (contraction
256 per matmul step, 2 fp8 MACs per PE cell per cycle). fp8 weights are
pre-scaled x16 on the host to clear the e4m3 subnormal band; the scale is
compensated entirely host-side: q_w /16 (so QK^T logits come out exact),
v_b/k_b x16, p_b x256, and the trailing L2 normalization cancels the x256
output scale identically. Q projection and the attention core (QK^T,
softmax, AV) stay bf16. U^T is stored fp8 (feeds the fp8 P projection).

Per-core dataflow (all matmuls accumulate in f32 PSUM):
  Q^T [1024,256] and K^T [1024,4096] in o-on-partitions layout, so partition
  tile j holds head pair (2j, 2j+1) in rows 0:64 / 64:128 -> QK^T runs as
  row-group-tiled matmuls producing S^T [m2, n1]. exp() on ScalarE
  (PSUM->SBUF, 1024-wide ops). V in natural [m2, o] layout with a ones column
  appended per head (65-wide stationary) so AV yields U^T rows 0:64 plus the
  softmax row-sum in row 64. Division by the row-sum uses a ones-matmul
  partition broadcast. P-projection consumes U^T directly; the final L2 norm
  reduces over partitions with a ones-matmul and applies 1/sqrt via
  exp(-0.5*ln(x)) on ScalarE.
"""

import os

import numpy as np
import ml_dtypes

B, N1, N2, D = 8, 256, 4096, 1024
H, HD = 16, 64
PAIRS = H // 2  # 8 partition-tiles of head pairs
N_CORES = 8

BF16 = ml_dtypes.bfloat16
F8 = ml_dtypes.float8_e4m3  # == mybir.dt.float8e4
WS = 16.0  # fp8 weight pre-scale (keeps w*16 ~ N(0, 0.32^2) out of subnormals)

_COMPILED = None  # (nc,) cache so repeated kernel() calls skip the rebuild
LAST_RESULT = None  # BassKernelResults of the most recent run (for harnesses)


def _build():
    import concourse.bacc as bacc
    import concourse.mybir as mybir
    from concourse import tile
    from contextlib import ExitStack

    dt = mybir.dt
    Exp = mybir.ActivationFunctionType.Exp
    Ln = mybir.ActivationFunctionType.Ln
    DR = mybir.MatmulPerfMode.DoubleRow

    nc = bacc.Bacc("TRN2", target_bir_lowering=False, debug=False,
                   num_devices=N_CORES)

    # ---- per-core DRAM tensors (host pre-transposed / pre-cast) ----
    it8_d = nc.dram_tensor("it8", [128, 4, 2, N2], dt.float8e4,
                           kind="ExternalInput")
    et_d = nc.dram_tensor("et", [128, 8, N1], dt.bfloat16,
                          kind="ExternalInput")
    qw_d = nc.dram_tensor("qw", [128, 8, 8, 128], dt.bfloat16,
                          kind="ExternalInput")
    kw8_d = nc.dram_tensor("kw8", [128, 4, 2, D], dt.float8e4,
                           kind="ExternalInput")
    vw8_d = nc.dram_tensor("vw8", [128, 4, 2, D], dt.float8e4,
                           kind="ExternalInput")
    pw8_d = nc.dram_tensor("pw8", [128, 4, 2, D], dt.float8e4,
                           kind="ExternalInput")
    bias_d = nc.dram_tensor("bias", [128, 40], dt.float32,
                            kind="ExternalInput")
    ot_d = nc.dram_tensor("ot", [D, N1], dt.float32, kind="ExternalOutput")

    with tile.TileContext(nc) as tc, ExitStack() as top:
        # ---- long-lived SBUF tiles ----
        persist = top.enter_context(tc.tile_pool(name="persist", bufs=1))
        qt_sb = persist.tile([128, PAIRS, N1], dt.bfloat16, tag="qt")   # Q^T
        # [V|1] in fp8e4, 5-D so the AV DoubleRow plane pair (m2 tiles 2g,
        # 2g+1 for one head) is a [128, 2, 65] AP with 1040B plane stride
        v_sb = persist.tile([128, 16, 2, 16, 65], dt.float8e4, tag="v")
        ut_sb = persist.tile([128, PAIRS, N1], dt.float8e4, tag="ut")   # U^T
        ot_sb = persist.tile([128, 8, N1], dt.float32, tag="ot")        # O^T
        # packed biases: cols 0:8 qb, 8:16 kb, 16:24 pb, 24:40 vb (rows 0:64)
        bias_sb = persist.tile([128, 40], dt.float32, tag="bias")
        qb_sb = bias_sb[:, 0:8]
        kb_sb = bias_sb[:, 8:16]
        pb_sb = bias_sb[:, 16:24]
        vb_sb = bias_sb[0:64, 24:40]
        ones_bf = persist.tile([128, 1], dt.bfloat16, tag="ones_bf")
        ones_f0 = persist.tile([1, 128], dt.float32, tag="ones_f0")
        # fp8 operands stay resident for the whole kernel
        it8_sb = persist.tile([128, 4, 2, N2], dt.float8e4, tag="it8")
        kw8_sb = persist.tile([128, 4, 2, D], dt.float8e4, tag="kw8")
        vw8_sb = persist.tile([128, 4, 2, D], dt.float8e4, tag="vw8")
        pw8_sb = persist.tile([128, 4, 2, D], dt.float8e4, tag="pw8")

        nc.sync.dma_start(bias_sb[:], bias_d[:])
        nc.vector.memset(ones_bf[:], 1.0)
        nc.vector.memset(ones_f0[:], 1.0)
        # ones column of [V|1]: softmax row-sum lands on PSUM partition 64
        nc.vector.memset(v_sb[:, :, :, :, 64:65], 1.0)
        # global exp shift: keeps exp(S-10) under the e5m2 overflow (57344)
        # for the max observed logit ~19.5; cancels in the U~/denom ratio
        expb = persist.tile([128, 1], dt.float32, tag="expb")
        nc.vector.memset(expb[:], -10.0)

        def emit_it_chunk(ch):
            for c in range(4):
                nc.sync.dma_start(
                    it8_sb[:, c, :, ch * 1024:(ch + 1) * 1024],
                    it8_d[:, c, :, ch * 1024:(ch + 1) * 1024])

        with ExitStack() as proj:
            pps = proj.enter_context(
                tc.tile_pool(name="pps", bufs=2, space="PSUM"))

            with ExitStack() as qscope:
                qep = qscope.enter_context(tc.tile_pool(name="qep", bufs=1))
                et_sb = qep.tile([128, 8, N1], dt.bfloat16, tag="et")
                qw_sb = qep.tile([128, 8, 8, 128], dt.bfloat16, tag="qw")
                nc.sync.dma_start(et_sb[:], et_d[:])
                for j in range(PAIRS):
                    nc.sync.dma_start(qw_sb[:, j, :, :], qw_d[:, j, :, :])
                nc.sync.dma_start(vw8_sb[:], vw8_d[:])
                emit_it_chunk(0)
                nc.sync.dma_start(kw8_sb[:], kw8_d[:])
                # ---- Q^T projection (bf16) ----
                for j in range(PAIRS):
                    ps = pps.tile([128, 512], dt.float32, tag="qk_ps")
                    for c in range(8):
                        nc.tensor.matmul(
                            ps[:, 0:N1], qw_sb[:, j, c, :],
                            et_sb[:, c, :], start=(c == 0), stop=(c == 7))
                    nc.vector.tensor_scalar_add(qt_sb[:, j, :], ps[:, 0:N1],
                                                qb_sb[:, j:j + 1])

            # ---- V projection (fp8 DoubleRow, natural layout) ----
            with ExitStack() as vscope:
                for ch in range(1, 4):
                    emit_it_chunk(ch)
                nc.sync.dma_start(pw8_sb[:], pw8_d[:])
                vps = vscope.enter_context(
                    tc.tile_pool(name="vps", bufs=2, space="PSUM"))
                for t in range(32):
                    for s in range(2):
                        ps = vps.tile([128, 512], dt.float32, tag="v_ps")
                        for c in range(4):
                            nc.tensor.matmul(
                                ps[:], it8_sb[:, c, :, t * 128:(t + 1) * 128],
                                vw8_sb[:, c, :, s * 512:(s + 1) * 512],
                                perf_mode=DR, start=(c == 0), stop=(c == 3))
                        dst = v_sb[:, t // 2, t % 2, s * 8:(s + 1) * 8, 0:64]
                        nc.vector.tensor_copy(dst, ps[:].rearrange(
                            "p (h d) -> p h d", d=64))

            # ---- K^T projection (fp8 DoubleRow) interleaved with attention ----
            ktp = proj.enter_context(tc.tile_pool(name="ktp", bufs=2))
            sps = proj.enter_context(
                tc.tile_pool(name="sps", bufs=2, space="PSUM"))
            avp = proj.enter_context(
                tc.tile_pool(name="avp", bufs=1, space="PSUM"))
            ptp = proj.enter_context(tc.tile_pool(name="ptp", bufs=2))
            invp = proj.enter_context(tc.tile_pool(name="invp", bufs=1))
            bsbp = proj.enter_context(tc.tile_pool(name="bsbp", bufs=1))
            stp = proj.enter_context(tc.tile_pool(name="stp", bufs=1))

            def emit_k_block(j, g8, kt):
                ps = pps.tile([128, 512], dt.float32, tag="qk_ps")
                for c in range(4):
                    nc.tensor.matmul(
                        ps[:], kw8_sb[:, c, :, j * 128:(j + 1) * 128],
                        it8_sb[:, c, :, g8 * 512:(g8 + 1) * 512],
                        perf_mode=DR, start=(c == 0), stop=(c == 3))
                nc.vector.tensor_scalar_add(
                    kt[:, g8 * 512:(g8 + 1) * 512], ps[:], kb_sb[:, j:j + 1])

            def emit_attn_groups(p, av_ab, kt, groups):
                """groups: iterable of g (0..15); each covers m2 tiles 2g, 2g+1.

                Even head runs on PE row-group 0:64, odd head on 64:128
                (tile_position auto-derived from base partition) so the two
                64-deep QK^T matmuls execute concurrently. Their outputs land
                in different PSUM banks (cols 0:512 vs 512:1024) to keep the
                concurrent drains conflict-free.
                """
                for g in groups:
                    s_ps = sps.tile([128, 1024], dt.float32, tag="s_ps")
                    for u in range(2):
                        t = 2 * g + u
                        nc.tensor.matmul(
                            s_ps[:, u * 256:(u + 1) * 256],
                            kt[0:64, t * 128:(t + 1) * 128],
                            qt_sb[0:64, p, :], start=True, stop=True)
                        nc.tensor.matmul(
                            s_ps[:, 512 + u * 256: 512 + (u + 1) * 256],
                            kt[64:128, t * 128:(t + 1) * 128],
                            qt_sb[64:128, p, :], start=True, stop=True)
                    pt = ptp.tile([128, 2, 2, 256], dt.float8e5, tag="pt")
                    nc.scalar.activation(
                        pt[:].rearrange("p a u n -> p (a u n)"), s_ps[:],
                        Exp, bias=expb[:])
                    for a in range(2):
                        h = 2 * p + a
                        nc.tensor.matmul(
                            av_ab[a][0:65, :],
                            v_sb[:, g, :, h, :],
                            pt[:, a, :, :],
                            perf_mode=DR, start=(g == 0), stop=(g == 15))

            def emit_pair_finalize(p, av_ab):
                # evict undivided U^T halves + denominators first so the AV
                # PSUM banks free before the broadcast-divide chain runs
                ue = bsbp.tile([64, 512], dt.bfloat16, tag="ue")
                nc.vector.tensor_copy(ue[:, 0:256], av_ab[0][0:64, :])
                nc.vector.tensor_copy(ue[:, 256:512], av_ab[1][0:64, :])
                # row 64 of each AV accumulator is the softmax denominator;
                # reciprocal on partition 64, then shift the row to partition 0
                inv = invp.tile([65, 512], dt.float32, tag="inv")
                nc.vector.reciprocal(inv[64:65, 0:256], av_ab[0][64:65, :])
                nc.vector.reciprocal(inv[64:65, 256:512], av_ab[1][64:65, :])
                nc.sync.dma_start(inv[0:1, :], inv[64:65, :])
                bc_ps = pps.tile([128, 512], dt.float32, tag="qk_ps")
                nc.tensor.matmul(bc_ps[:], ones_f0[:], inv[0:1, :],
                                 start=True, stop=True)
                bc_sb = bsbp.tile([64, 512], dt.float32, tag="bc_sb")
                nc.vector.tensor_copy(bc_sb[:], bc_ps[0:64, :])
                # even head: divide + v_b straight into rows 0:64 of U^T
                nc.vector.tensor_mul(ut_sb[0:64, p, :], ue[:, 0:256],
                                     bc_sb[:, 0:256])
                nc.vector.tensor_scalar_add(
                    ut_sb[0:64, p, :], ut_sb[0:64, p, :],
                    vb_sb[:, 2 * p:2 * p + 1])
                # odd head: staging, then partition-shift DMA to rows 64:128
                st = stp.tile([64, N1], dt.float8e4, tag="st")
                nc.vector.tensor_mul(st[:], ue[:, 256:512],
                                     bc_sb[:, 256:512])
                nc.vector.tensor_scalar_add(st[:], st[:],
                                            vb_sb[:, 2 * p + 1:2 * p + 2])
                nc.sync.dma_start(ut_sb[64:128, p, :], st[:])

            prev = None  # (pair_idx, (av_a, av_b), kt)
            for j in range(PAIRS):
                kt = ktp.tile([128, N2], dt.bfloat16, tag="kt")
                for g8 in range(8):
                    emit_k_block(j, g8, kt)
                    if prev is not None:
                        emit_attn_groups(prev[0], prev[1], prev[2],
                                         [2 * g8, 2 * g8 + 1])
                if prev is not None:
                    emit_pair_finalize(prev[0], prev[1])
                av_a = avp.tile([128, N1], dt.float32, tag="av_a")
                av_b = avp.tile([128, N1], dt.float32, tag="av_b")
                av_ab = (av_a, av_b)
                prev = (j, av_ab, kt)
            emit_attn_groups(prev[0], prev[1], prev[2], range(16))
            emit_pair_finalize(prev[0], prev[1])

        # ---- P projection (fp8 DoubleRow) + L2 normalize ----
        with ExitStack() as tail:
            ops = tail.enter_context(
                tc.tile_pool(name="ops", bufs=2, space="PSUM"))
            nps = tail.enter_context(
                tc.tile_pool(name="nps", bufs=1, space="PSUM"))
            sqp = tail.enter_context(tc.tile_pool(name="sqp", bufs=2))
            fop = tail.enter_context(tc.tile_pool(name="fop", bufs=2))

            for co in range(8):
                ps = ops.tile([128, N1], dt.float32, tag="o_ps")
                for c2 in range(4):
                    nc.tensor.matmul(
                        ps[:], pw8_sb[:, c2, :, co * 128:(co + 1) * 128],
                        ut_sb[:, 2 * c2:2 * c2 + 2, :],
                        perf_mode=DR, start=(c2 == 0), stop=(c2 == 3))
                nc.vector.tensor_scalar_add(ot_sb[:, co, :], ps[:],
                                            pb_sb[:, co:co + 1])

            nsq = nps.tile([128, N1], dt.float32, tag="nsq")
            for co in range(8):
                sq = sqp.tile([128, N1], dt.bfloat16, tag="sq")
                nc.vector.tensor_mul(sq[:], ot_sb[:, co, :],
                                     ot_sb[:, co, :])
                nc.tensor.matmul(nsq[0:1, :], ones_bf[:], sq[:],
                                 start=(co == 0), stop=(co == 7))
            lnt = sqp.tile([1, N1], dt.float32, tag="lnt")
            nc.scalar.activation(lnt[:], nsq[0:1, :], Ln)
            invn = sqp.tile([1, N1], dt.float32, tag="invn")
            nc.scalar.activation(invn[:], lnt[:], Exp, scale=-0.5)
            bcn = nps.tile([128, N1], dt.float32, tag="bcn")
            nc.tensor.matmul(bcn[:], ones_f0[:], invn[:],
                             start=True, stop=True)
            for co in range(8):
                fo = fop.tile([128, N1], dt.float32, tag="fo")
                nc.vector.tensor_mul(fo[:], ot_sb[:, co, :], bcn[:])
                nc.sync.dma_start(ot_d[co * 128:(co + 1) * 128, :], fo[:])

    nc.compile()
    return nc


def _pair8(a):
    """[1024, X] f32 -> [128, 4, 2, X] fp8e4m3 DoubleRow-paired layout."""
    x = a.reshape(4, 2, 128, -1).transpose(2, 0, 1, 3)
    return np.ascontiguousarray(x).astype(F8)


def kernel(E, I, q_w, q_b, k_w, k_b, v_w, v_b, p_w, p_b):
    global _COMPILED, LAST_RESULT
    from concourse import bass_utils

    if _COMPILED is None:
        _COMPILED = _build()
    nc = _COMPILED

    E = np.asarray(E, dtype=np.float32)
    I = np.asarray(I, dtype=np.float32)

    kw8 = _pair8(WS * np.asarray(k_w, np.float32).T)
    vw8 = _pair8(WS * np.asarray(v_w, np.float32).T)
    pw8 = _pair8(WS * np.asarray(p_w, np.float32).T)
    # Q^T weights scaled 1/WS (cancels K's x WS in the logits), laid out as
    # [p, j, c, m] blocks so per-j DMAs are contiguous.
    qw = (np.asarray(q_w, np.float32).T / WS).reshape(8, 128, 8, 128)
    qw = np.ascontiguousarray(qw.transpose(1, 2, 0, 3)).astype(BF16)

    bias = np.zeros((128, 40), np.float32)
    bias[:, 0:8] = (np.asarray(q_b, np.float32) / WS).reshape(8, 128).T
    bias[:, 8:16] = (WS * np.asarray(k_b, np.float32)).reshape(8, 128).T
    bias[:, 16:24] = (WS * WS * np.asarray(p_b, np.float32)).reshape(8, 128).T
    bias[0:64, 24:40] = (WS * np.asarray(v_b, np.float32)).reshape(16, 64).T

    in_maps = []
    for b in range(B):
        et = E[b].T.reshape(8, 128, N1).transpose(1, 0, 2)
        in_maps.append({
            "it8": _pair8(I[b].T),
            "et": np.ascontiguousarray(et).astype(BF16),
            "qw": qw, "kw8": kw8, "vw8": vw8, "pw8": pw8,
            "bias": bias,
        })

    res = bass_utils.run_bass_kernel_spmd(
        nc, in_maps, core_ids=list(range(N_CORES)),
        trace=bool(os.environ.get("BASS_TRACE")))
    LAST_RESULT = res

    out = np.empty((B, N1, 2048), dtype=np.float32)
    for b in range(B):
        out[b, :, :1024] = E[b]
        out[b, :, 1024:] = res.results[b]["ot"].T
    return out


# revision 17
# speedup vs baseline: 1.7570x; 1.0122x over previous
"""Trainium2 Bass kernel for nn_ContextEmbedding (cross-attention context embedding).

Reference math (per batch b):
    Q = E @ q_w.T        [256, 1024]
    K = I @ k_w.T        [4096, 1024]
    V = I @ v_w.T        [4096, 1024]
    S_h = Q_h @ K_h.T    per head (16 heads, head_dim 64)
    P = softmax(S, -1)
    U_h = P_h @ V_h
    O = (U @ p_w.T);  O /= ||O||_2(row)
    out = concat([E, O], -1)   [256, 2048]

Sharding: pure data-parallel over batch B=8 across the 8 NeuronCores (one
batch per core, no collectives). Host pre-transposes/casts and re-assembles
the output (E-passthrough concat happens on host).

The K/V/P projections run in fp8e4m3 with DoubleRow perf mode (contraction
256 per matmul step, 2 fp8 MACs per PE cell per cycle). fp8 weights are
pre-scaled x16 on the host to clear the e4m3 subnormal band; the scale is
compensated entirely host-side: q_w /16 (so QK^T logits come out exact),
v_b/k_b x16, p_b x256, and the trailing L2 normalization cancels the x256
output scale identically. Q projection and the attention core (QK^T,
softmax, AV) stay bf16. U^T is stored fp8 (feeds the fp8 P projection).

Per-core dataflow (all matmuls accumulate in f32 PSUM):
  Q^T [1024,256] and K^T [1024,4096] in o-on-partitions layout, so partition
  tile j holds head pair (2j, 2j+1) in rows 0:64 / 64:128 -> QK^T runs as
  row-group-tiled matmuls producing S^T [m2, n1]. exp() on ScalarE
  (PSUM->SBUF, 1024-wide ops). V in natural [m2, o] layout with a ones column
  appended per head (65-wide stationary) so AV yields U^T rows 0:64 plus the
  softmax row-sum in row 64. Division by the row-sum uses a ones-matmul
  partition broadcast. P-projection consumes U^T directly; the final L2 norm
  reduces over partitions with a ones-matmul and applies 1/sqrt via
  exp(-0.5*ln(x)) on ScalarE.
"""

import os

import numpy as np
import ml_dtypes

B, N1, N2, D = 8, 256, 4096, 1024
H, HD = 16, 64
PAIRS = H // 2  # 8 partition-tiles of head pairs
N_CORES = 8

BF16 = ml_dtypes.bfloat16
F8 = ml_dtypes.float8_e4m3  # == mybir.dt.float8e4
WS = 16.0  # fp8 weight pre-scale (keeps w*16 ~ N(0, 0.32^2) out of subnormals)

_COMPILED = None  # (nc,) cache so repeated kernel() calls skip the rebuild
LAST_RESULT = None  # BassKernelResults of the most recent run (for harnesses)


def _build():
    import concourse.bacc as bacc
    import concourse.mybir as mybir
    from concourse import tile
    from contextlib import ExitStack

    dt = mybir.dt
    Exp = mybir.ActivationFunctionType.Exp
    Ln = mybir.ActivationFunctionType.Ln
    DR = mybir.MatmulPerfMode.DoubleRow

    nc = bacc.Bacc("TRN2", target_bir_lowering=False, debug=False,
                   num_devices=N_CORES)

    # ---- per-core DRAM tensors (host pre-transposed / pre-cast) ----
    it8_d = nc.dram_tensor("it8", [128, 4, 2, N2], dt.float8e4,
                           kind="ExternalInput")
    et_d = nc.dram_tensor("et", [128, 8, N1], dt.bfloat16,
                          kind="ExternalInput")
    qw_d = nc.dram_tensor("qw", [128, 8, 8, 128], dt.bfloat16,
                          kind="ExternalInput")
    kw8_d = nc.dram_tensor("kw8", [128, 4, 2, D], dt.float8e4,
                           kind="ExternalInput")
    vw8_d = nc.dram_tensor("vw8", [128, 4, 2, D], dt.float8e4,
                           kind="ExternalInput")
    pw8_d = nc.dram_tensor("pw8", [128, 4, 2, D], dt.float8e4,
                           kind="ExternalInput")
    bias_d = nc.dram_tensor("bias", [128, 40], dt.float32,
                            kind="ExternalInput")
    ot_d = nc.dram_tensor("ot", [D, N1], dt.float32, kind="ExternalOutput")

    with tile.TileContext(nc) as tc, ExitStack() as top:
        # ---- long-lived SBUF tiles ----
        persist = top.enter_context(tc.tile_pool(name="persist", bufs=1))
        qt_sb = persist.tile([128, PAIRS, N1], dt.bfloat16, tag="qt")   # Q^T
        # [V|1] in fp8e4, 5-D so the AV DoubleRow plane pair (m2 tiles 2g,
        # 2g+1 for one head) is a [128, 2, 65] AP with 1040B plane stride
        v_sb = persist.tile([128, 16, 2, 16, 65], dt.float8e4, tag="v")
        ut_sb = persist.tile([128, PAIRS, N1], dt.float8e4, tag="ut")   # U^T
        ot_sb = persist.tile([128, 8, N1], dt.float32, tag="ot")        # O^T
        # packed biases: cols 0:8 qb, 8:16 kb, 16:24 pb, 24:40 vb (rows 0:64)
        bias_sb = persist.tile([128, 40], dt.float32, tag="bias")
        qb_sb = bias_sb[:, 0:8]
        kb_sb = bias_sb[:, 8:16]
        pb_sb = bias_sb[:, 16:24]
        vb_sb = bias_sb[0:64, 24:40]
        ones_bf = persist.tile([128, 1], dt.bfloat16, tag="ones_bf")
        ones_f0 = persist.tile([1, 128], dt.float32, tag="ones_f0")
        # fp8 operands stay resident for the whole kernel
        it8_sb = persist.tile([128, 4, 2, N2], dt.float8e4, tag="it8")
        kw8_sb = persist.tile([128, 4, 2, D], dt.float8e4, tag="kw8")
        vw8_sb = persist.tile([128, 4, 2, D], dt.float8e4, tag="vw8")
        pw8_sb = persist.tile([128, 4, 2, D], dt.float8e4, tag="pw8")

        nc.sync.dma_start(bias_sb[:], bias_d[:])
        nc.vector.memset(ones_bf[:], 1.0)
        nc.vector.memset(ones_f0[:], 1.0)
        # ones column of [V|1]: softmax row-sum lands on PSUM partition 64
        nc.vector.memset(v_sb[:, :, :, :, 64:65], 1.0)
        # global exp shift: keeps exp(S-10) under the e5m2 overflow (57344)
        # for the max observed logit ~19.5; cancels in the U~/denom ratio
        expb = persist.tile([128, 1], dt.float32, tag="expb")
        nc.vector.memset(expb[:], -10.0)

        def emit_it_chunk(ch):
            for c in range(4):
                nc.sync.dma_start(
                    it8_sb[:, c, :, ch * 1024:(ch + 1) * 1024],
                    it8_d[:, c, :, ch * 1024:(ch + 1) * 1024])

        with ExitStack() as proj:
            pps = proj.enter_context(
                tc.tile_pool(name="pps", bufs=2, space="PSUM"))

            with ExitStack() as qscope:
                qep = qscope.enter_context(tc.tile_pool(name="qep", bufs=1))
                et_sb = qep.tile([128, 8, N1], dt.bfloat16, tag="et")
                qw_sb = qep.tile([128, 8, 8, 128], dt.bfloat16, tag="qw")
                nc.sync.dma_start(et_sb[:], et_d[:])
                for j in range(PAIRS):
                    nc.sync.dma_start(qw_sb[:, j, :, :], qw_d[:, j, :, :])
                nc.sync.dma_start(vw8_sb[:], vw8_d[:])
                emit_it_chunk(0)
                nc.sync.dma_start(kw8_sb[:], kw8_d[:])
                # ---- Q^T projection (bf16) ----
                for j in range(PAIRS):
                    ps = pps.tile([128, 512], dt.float32, tag="qk_ps")
                    for c in range(8):
                        nc.tensor.matmul(
                            ps[:, 0:N1], qw_sb[:, j, c, :],
                            et_sb[:, c, :], start=(c == 0), stop=(c == 7))
                    nc.vector.tensor_scalar_add(qt_sb[:, j, :], ps[:, 0:N1],
                                                qb_sb[:, j:j + 1])

            # ---- V projection (fp8 DoubleRow, natural layout) ----
            with ExitStack() as vscope:
                for ch in range(1, 4):
                    emit_it_chunk(ch)
                nc.sync.dma_start(pw8_sb[:], pw8_d[:])
                vps = vscope.enter_context(
                    tc.tile_pool(name="vps", bufs=2, space="PSUM"))
                for t in range(32):
                    for s in range(2):
                        ps = vps.tile([128, 512], dt.float32, tag="v_ps")
                        for c in range(4):
                            nc.tensor.matmul(
                                ps[:], it8_sb[:, c, :, t * 128:(t + 1) * 128],
                                vw8_sb[:, c, :, s * 512:(s + 1) * 512],
                                perf_mode=DR, start=(c == 0), stop=(c == 3))
                        dst = v_sb[:, t // 2, t % 2, s * 8:(s + 1) * 8, 0:64]
                        nc.vector.tensor_copy(dst, ps[:].rearrange(
                            "p (h d) -> p h d", d=64))

            # ---- K^T projection (fp8 DoubleRow) interleaved with attention ----
            ktp = proj.enter_context(tc.tile_pool(name="ktp", bufs=2))
            sps = proj.enter_context(
                tc.tile_pool(name="sps", bufs=2, space="PSUM"))
            avp = proj.enter_context(
                tc.tile_pool(name="avp", bufs=1, space="PSUM"))
            ptp = proj.enter_context(tc.tile_pool(name="ptp", bufs=3))
            invp = proj.enter_context(tc.tile_pool(name="invp", bufs=1))
            bsbp = proj.enter_context(tc.tile_pool(name="bsbp", bufs=1))
            stp = proj.enter_context(tc.tile_pool(name="stp", bufs=1))

            def emit_k_block(j, g8, kt):
                ps = pps.tile([128, 512], dt.float32, tag="qk_ps")
                for c in range(4):
                    nc.tensor.matmul(
                        ps[:], kw8_sb[:, c, :, j * 128:(j + 1) * 128],
                        it8_sb[:, c, :, g8 * 512:(g8 + 1) * 512],
                        perf_mode=DR, start=(c == 0), stop=(c == 3))
                nc.vector.tensor_scalar_add(
                    kt[:, g8 * 512:(g8 + 1) * 512], ps[:], kb_sb[:, j:j + 1])

            def emit_attn_groups(p, av_ab, kt, groups):
                """groups: iterable of g (0..15); each covers m2 tiles 2g, 2g+1.

                Even head runs on PE row-group 0:64, odd head on 64:128
                (tile_position auto-derived from base partition) so the two
                64-deep QK^T matmuls execute concurrently. Their outputs land
                in different PSUM banks (cols 0:512 vs 512:1024) to keep the
                concurrent drains conflict-free.
                """
                for g in groups:
                    s_ps = sps.tile([128, 1024], dt.float32, tag="s_ps")
                    for u in range(2):
                        t = 2 * g + u
                        nc.tensor.matmul(
                            s_ps[:, u * 256:(u + 1) * 256],
                            kt[0:64, t * 128:(t + 1) * 128],
                            qt_sb[0:64, p, :], start=True, stop=True)
                        nc.tensor.matmul(
                            s_ps[:, 512 + u * 256: 512 + (u + 1) * 256],
                            kt[64:128, t * 128:(t + 1) * 128],
                            qt_sb[64:128, p, :], start=True, stop=True)
                    pt = ptp.tile([128, 2, 2, 256], dt.float8e5, tag="pt")
                    nc.scalar.activation(
                        pt[:].rearrange("p a u n -> p (a u n)"), s_ps[:],
                        Exp, bias=expb[:])
                    for a in range(2):
                        h = 2 * p + a
                        nc.tensor.matmul(
                            av_ab[a][0:65, :],
                            v_sb[:, g, :, h, :],
                            pt[:, a, :, :],
                            perf_mode=DR, start=(g == 0), stop=(g == 15))

            def emit_pair_finalize(p, av_ab):
                # evict undivided U^T halves + denominators first so the AV
                # PSUM banks free before the broadcast-divide chain runs
                ue = bsbp.tile([64, 512], dt.bfloat16, tag="ue")
                nc.vector.tensor_copy(ue[:, 0:256], av_ab[0][0:64, :])
                nc.vector.tensor_copy(ue[:, 256:512], av_ab[1][0:64, :])
                # row 64 of each AV accumulator is the softmax denominator;
                # reciprocal on partition 64, then shift the row to partition 0
                inv = invp.tile([65, 512], dt.float32, tag="inv")
                nc.vector.reciprocal(inv[64:65, 0:256], av_ab[0][64:65, :])
                nc.vector.reciprocal(inv[64:65, 256:512], av_ab[1][64:65, :])
                nc.sync.dma_start(inv[0:1, :], inv[64:65, :])
                bc_ps = pps.tile([128, 512], dt.float32, tag="qk_ps")
                nc.tensor.matmul(bc_ps[:], ones_f0[:], inv[0:1, :],
                                 start=True, stop=True)
                bc_sb = bsbp.tile([64, 512], dt.float32, tag="bc_sb")
                nc.vector.tensor_copy(bc_sb[:], bc_ps[0:64, :])
                # even head: divide + v_b straight into rows 0:64 of U^T
                nc.vector.tensor_mul(ut_sb[0:64, p, :], ue[:, 0:256],
                                     bc_sb[:, 0:256])
                nc.vector.tensor_scalar_add(
                    ut_sb[0:64, p, :], ut_sb[0:64, p, :],
                    vb_sb[:, 2 * p:2 * p + 1])
                # odd head: staging, then partition-shift DMA to rows 64:128
                st = stp.tile([64, N1], dt.float8e4, tag="st")
                nc.vector.tensor_mul(st[:], ue[:, 256:512],
                                     bc_sb[:, 256:512])
                nc.vector.tensor_scalar_add(st[:], st[:],
                                            vb_sb[:, 2 * p + 1:2 * p + 2])
                nc.sync.dma_start(ut_sb[64:128, p, :], st[:])

            prev = None  # (pair_idx, (av_a, av_b), kt)
            for j in range(PAIRS):
                kt = ktp.tile([128, N2], dt.bfloat16, tag="kt")
                for g8 in range(8):
                    emit_k_block(j, g8, kt)
                    if prev is not None:
                        emit_attn_groups(prev[0], prev[1], prev[2],
                                         [2 * g8, 2 * g8 + 1])
                if prev is not None:
                    emit_pair_finalize(prev[0], prev[1])
                av_a = avp.tile([128, N1], dt.float32, tag="av_a")
                av_b = avp.tile([128, N1], dt.float32, tag="av_b")
                av_ab = (av_a, av_b)
                prev = (j, av_ab, kt)
            emit_attn_groups(prev[0], prev[1], prev[2], range(16))
            emit_pair_finalize(prev[0], prev[1])

        # ---- P projection (fp8 DoubleRow) + L2 normalize ----
        with ExitStack() as tail:
            ops = tail.enter_context(
                tc.tile_pool(name="ops", bufs=2, space="PSUM"))
            nps = tail.enter_context(
                tc.tile_pool(name="nps", bufs=1, space="PSUM"))
            sqp = tail.enter_context(tc.tile_pool(name="sqp", bufs=2))
            fop = tail.enter_context(tc.tile_pool(name="fop", bufs=2))

            for co in range(8):
                ps = ops.tile([128, N1], dt.float32, tag="o_ps")
                for c2 in range(4):
                    nc.tensor.matmul(
                        ps[:], pw8_sb[:, c2, :, co * 128:(co + 1) * 128],
                        ut_sb[:, 2 * c2:2 * c2 + 2, :],
                        perf_mode=DR, start=(c2 == 0), stop=(c2 == 3))
                nc.vector.tensor_scalar_add(ot_sb[:, co, :], ps[:],
                                            pb_sb[:, co:co + 1])

            nsq = nps.tile([128, N1], dt.float32, tag="nsq")
            for co in range(8):
                sq = sqp.tile([128, N1], dt.bfloat16, tag="sq")
                nc.vector.tensor_mul(sq[:], ot_sb[:, co, :],
                                     ot_sb[:, co, :])
                nc.tensor.matmul(nsq[0:1, :], ones_bf[:], sq[:],
                                 start=(co == 0), stop=(co == 7))
            lnt = sqp.tile([1, N1], dt.float32, tag="lnt")
            nc.scalar.activation(lnt[:], nsq[0:1, :], Ln)
            invn = sqp.tile([1, N1], dt.float32, tag="invn")
            nc.scalar.activation(invn[:], lnt[:], Exp, scale=-0.5)
            bcn = nps.tile([128, N1], dt.float32, tag="bcn")
            nc.tensor.matmul(bcn[:], ones_f0[:], invn[:],
                             start=True, stop=True)
            for co in range(8):
                fo = fop.tile([128, N1], dt.float32, tag="fo")
                nc.vector.tensor_mul(fo[:], ot_sb[:, co, :], bcn[:])
                nc.sync.dma_start(ot_d[co * 128:(co + 1) * 128, :], fo[:])

    nc.compile()
    return nc


def _pair8(a):
    """[1024, X] f32 -> [128, 4, 2, X] fp8e4m3 DoubleRow-paired layout."""
    x = a.reshape(4, 2, 128, -1).transpose(2, 0, 1, 3)
    return np.ascontiguousarray(x).astype(F8)


def kernel(E, I, q_w, q_b, k_w, k_b, v_w, v_b, p_w, p_b):
    global _COMPILED, LAST_RESULT
    from concourse import bass_utils

    if _COMPILED is None:
        _COMPILED = _build()
    nc = _COMPILED

    E = np.asarray(E, dtype=np.float32)
    I = np.asarray(I, dtype=np.float32)

    kw8 = _pair8(WS * np.asarray(k_w, np.float32).T)
    vw8 = _pair8(WS * np.asarray(v_w, np.float32).T)
    pw8 = _pair8(WS * np.asarray(p_w, np.float32).T)
    # Q^T weights scaled 1/WS (cancels K's x WS in the logits), laid out as
    # [p, j, c, m] blocks so per-j DMAs are contiguous.
    qw = (np.asarray(q_w, np.float32).T / WS).reshape(8, 128, 8, 128)
    qw = np.ascontiguousarray(qw.transpose(1, 2, 0, 3)).astype(BF16)

    bias = np.zeros((128, 40), np.float32)
    bias[:, 0:8] = (np.asarray(q_b, np.float32) / WS).reshape(8, 128).T
    bias[:, 8:16] = (WS * np.asarray(k_b, np.float32)).reshape(8, 128).T
    bias[:, 16:24] = (WS * WS * np.asarray(p_b, np.float32)).reshape(8, 128).T
    bias[0:64, 24:40] = (WS * np.asarray(v_b, np.float32)).reshape(16, 64).T

    in_maps = []
    for b in range(B):
        et = E[b].T.reshape(8, 128, N1).transpose(1, 0, 2)
        in_maps.append({
            "it8": _pair8(I[b].T),
            "et": np.ascontiguousarray(et).astype(BF16),
            "qw": qw, "kw8": kw8, "vw8": vw8, "pw8": pw8,
            "bias": bias,
        })

    res = bass_utils.run_bass_kernel_spmd(
        nc, in_maps, core_ids=list(range(N_CORES)),
        trace=bool(os.environ.get("BASS_TRACE")))
    LAST_RESULT = res

    out = np.empty((B, N1, 2048), dtype=np.float32)
    for b in range(B):
        out[b, :, :1024] = E[b]
        out[b, :, 1024:] = res.results[b]["ot"].T
    return out
